# revision 1
# baseline (speedup 1.0000x reference)
"""GPT2 attention (B=2, S=2048, E=1024, H=16) on 8 NeuronCores.

Sharding: tensor-parallel over heads — 2 heads per core. Each core computes
qkv^T for its heads, causal attention in transposed-score layout (k on
partitions, q on free dim), then a partial output projection over its 128
ctx dims. Host sums the 8 partials and adds b_proj.

Compute is bf16 (f32 PSUM accumulation); validated rel-l2 ~4e-3 vs the f32
reference. Causal structure: only lower-triangular 128x512 score blocks are
computed; diagonal blocks are masked via gpsimd affine_select after exp.
Softmax denominator comes free from a ones-column appended to V (PV matmul
row 64); normalization is a rank-1 reciprocal broadcast matmul + DVE mul.
"""
import os
import numpy as np
import ml_dtypes

import concourse.bass as bass
import concourse.bacc as bacc
import concourse.tile as tile
from concourse import mybir
from concourse import masks
from concourse.bass_utils import run_bass_kernel_spmd

BF16 = ml_dtypes.bfloat16
B, S, E, H, D = 2, 2048, 1024, 16, 64
T = B * S                 # 4096 tokens
NCORE = 8
HPC = H // NCORE          # 2 heads per core
NEG = -10000.0
SCALE = D ** -0.5
F32 = mybir.dt.float32
BF = mybir.dt.bfloat16
EXP = mybir.ActivationFunctionType.Exp

_built = {}


def _build():
    if "nc" in _built:
        return _built["nc"]
    nc = bacc.Bacc()
    hsT = nc.declare_dram_parameter("hsT", [E, T], BF, isOutput=False)
    wqkv = nc.declare_dram_parameter("wqkv", [E, 3 * HPC * D], BF, isOutput=False)
    bqkv = nc.declare_dram_parameter("bqkv", [1, 3 * HPC * D], BF, isOutput=False)
    wpT = nc.declare_dram_parameter("wpT", [HPC * D, E], BF, isOutput=False)
    padneg = nc.declare_dram_parameter("padneg", [128, 32], F32, isOutput=False)
    out = nc.declare_dram_parameter("out", [T, E], BF, isOutput=True)

    NQ = S // 512             # 4 q-tiles of 512 per batch
    NK = S // 128             # 16 k-chunks of 128 per batch

    with tile.TileContext(nc) as tc:
        with (
            tc.tile_pool(name="const", bufs=1) as constp,
            tc.tile_pool(name="hst", bufs=8) as hstp,
            tc.tile_pool(name="big", bufs=1) as bigp,
            tc.tile_pool(name="expt", bufs=2) as exptp,
            tc.tile_pool(name="small", bufs=3) as smallp,
            tc.tile_pool(name="outp", bufs=4) as outp,
            tc.tile_pool(name="ps_qkv", bufs=4, space="PSUM") as ps_qkv,
            tc.tile_pool(name="ps_sc", bufs=3, space="PSUM") as ps_sc,
        ):
            # ---- constants ----
            wqkv_sb = constp.tile([128, 8, 384], BF)
            nc.sync.dma_start(
                out=wqkv_sb, in_=wqkv.rearrange("(kc p) m -> p kc m", p=128)
            )
            bq_sb = constp.tile([1, 384], BF)
            nc.sync.dma_start(out=bq_sb, in_=bqkv[:])
            wpT_sb = constp.tile([128, E], BF)
            nc.sync.dma_start(out=wpT_sb, in_=wpT[:])
            pad_sb = constp.tile([128, 32], F32)
            nc.sync.dma_start(out=pad_sb, in_=padneg[:])
            ident = constp.tile([128, 128], BF)
            masks.make_identity(nc, ident[:])
            ones_bf = constp.tile([1, 512], BF)
            nc.vector.memset(ones_bf, 1.0)
            ones64 = constp.tile([1, 64], F32)
            nc.vector.memset(ones64, 1.0)

            qT = bigp.tile([128, T], BF)       # rows: h0 dims 0-63, h1 dims 64-127
            kT = bigp.tile([128, T], BF)
            ctxT = bigp.tile([128, T], BF)
            # v in natural layout: per 128-token chunk tt, 130 cols:
            # [0:64]=h0 dims, [64]=ones, [65:129]=h1 dims, [129]=ones
            vnat = bigp.tile([128, 32, 130], BF)
            ctxn = bigp.tile([128, 32, 128], BF)
            nc.vector.memset(vnat[:, :, 64:65], 1.0)
            nc.vector.memset(vnat[:, :, 129:130], 1.0)

            hsT_r = hsT.rearrange("(kc p) t -> kc p t", p=128)  # [8,128,4096]

            # ---- phase B: qkv^T = Wc @ hsT + b, and v transpose ----
            for n in range(8):                  # 512-token tiles (batch0 first)
                pm = [ps_qkv.tile([128, 512], F32, tag="qkv", name=f"qkv{n}_{m}")
                      for m in range(3)]
                for m in range(3):
                    nc.tensor.matmul(
                        pm[m], lhsT=bq_sb[:, m * 128:(m + 1) * 128], rhs=ones_bf,
                        start=True, stop=False,
                    )
                for k in range(8):
                    ht = hstp.tile([128, 512], BF, tag="ht")
                    nc.sync.dma_start(out=ht, in_=hsT_r[k, :, n * 512:(n + 1) * 512])
                    for m in range(3):
                        nc.tensor.matmul(
                            pm[m], lhsT=wqkv_sb[:, k, m * 128:(m + 1) * 128], rhs=ht,
                            start=False, stop=(k == 7),
                        )
                nc.vector.tensor_copy(qT[:, n * 512:(n + 1) * 512], pm[0])
                nc.vector.tensor_copy(kT[:, n * 512:(n + 1) * 512], pm[1])
                vtmp = smallp.tile([128, 512], BF, tag="vtmp")
                nc.vector.tensor_copy(vtmp, pm[2])
                for t in range(4):
                    tt = n * 4 + t
                    pst = ps_sc.tile([128, 128], BF, tag="sc")
                    nc.tensor.transpose(pst[:], vtmp[:, t * 128:(t + 1) * 128], ident[:])
                    nc.vector.tensor_copy(vnat[:, tt, 0:64], pst[:, 0:64])
                    nc.vector.tensor_copy(vnat[:, tt, 65:129], pst[:, 64:128])

            # ---- phase C: causal attention ----
            # Scores transposed [k-part, q-free]; PV emits natural-layout ctx
            # [q-part, 65] per 128-q subtile (col 64 = softmax denominator from
            # the ones-column in vnat). Each qs accumulator gets its OWN psum
            # bank (concurrent accumulation groups must not share a bank).
            for b in range(B):
                for h in range(HPC):
                    hs_, he_ = h * 64, (h + 1) * 64
                    for qj in range(NQ):
                        nk = 4 * qj + 4
                        ctxq = [ps_qkv.tile([128, 512], F32, tag="qkv",
                                            name=f"cx{b}{h}{qj}_{qs}")
                                for qs in range(4)]
                        for ki in range(nk):
                            d = ki - 4 * qj
                            scp = ps_sc.tile([128, 512], F32, tag="sc",
                                             name=f"sc{b}{h}{qj}_{ki}")
                            nc.tensor.matmul(
                                scp,
                                lhsT=kT[hs_:he_, b * S + ki * 128: b * S + (ki + 1) * 128],
                                rhs=qT[hs_:he_, b * S + qj * 512: b * S + (qj + 1) * 512],
                                start=True, stop=True,
                            )
                            expt = exptp.tile([128, 512], BF, tag="expt",
                                              name=f"ex{b}{h}{qj}_{ki}")
                            c0 = 128 * d if d > 0 else 0
                            nc.scalar.activation(
                                out=expt[:, c0:], in_=scp[:, c0:], func=EXP,
                                bias=pad_sb[:, b * 16 + ki: b * 16 + ki + 1],
                                scale=SCALE,
                            )
                            if d >= 0:   # diagonal: zero where k > q (fills cols < c0 too)
                                nc.gpsimd.affine_select(
                                    out=expt, in_=expt,
                                    compare_op=mybir.AluOpType.is_ge, fill=0.0,
                                    base=-(128 * d), channel_multiplier=-1,
                                    pattern=[[1, 512]],
                                )
                            for qs in range(max(0, d), 4):
                                nc.tensor.matmul(
                                    ctxq[qs][:, 0:65],
                                    lhsT=expt[:, qs * 128:(qs + 1) * 128],
                                    rhs=vnat[:, b * 16 + ki, h * 65:(h + 1) * 65],
                                    start=(ki == 0), stop=(ki == 4 * qj + qs),
                                )
                        # normalize: col 64 is the denominator
                        recs = smallp.tile([128, 4], F32, tag="recs",
                                           name=f"rec{b}_{h}_{qj}")
                        for qs in range(4):
                            nc.vector.reciprocal(recs[:, qs:qs + 1],
                                                 ctxq[qs][:, 64:65])
                        for qs in range(4):
                            tt = b * 16 + qj * 4 + qs
                            nc.vector.tensor_scalar_mul(
                                ctxn[:, tt, hs_:he_], ctxq[qs][:, 0:64],
                                recs[:, qs:qs + 1])
                # transpose normalized ctx back to [dims, tokens] for proj
                for tl in range(16):
                    tt = b * 16 + tl
                    ctp = ps_sc.tile([128, 128], BF, tag="sc", name=f"ctp{b}_{tl}")
                    nc.tensor.transpose(ctp, ctxn[:, tt, :], ident[:])
                    nc.vector.tensor_copy(ctxT[:, tt * 128:(tt + 1) * 128], ctp)

            # ---- phase D: partial out projection ----
            for mt in range(32):
                for n2 in range(2):
                    pp = ps_qkv.tile([128, 512], F32, tag="qkv")
                    nc.tensor.matmul(
                        pp, lhsT=ctxT[:, mt * 128:(mt + 1) * 128],
                        rhs=wpT_sb[:, n2 * 512:(n2 + 1) * 512],
                        start=True, stop=True,
                    )
                    ot = outp.tile([128, 512], BF, tag="ot")
                    nc.vector.tensor_copy(ot, pp)
                    nc.sync.dma_start(
                        out=out[mt * 128:(mt + 1) * 128, n2 * 512:(n2 + 1) * 512],
                        in_=ot,
                    )
    nc.finalize()
    _built["nc"] = nc
    return nc


def kernel(hidden_states, attention_mask, W_attn, b_attn, W_proj, b_proj,
           _trace=False):
    hs = np.asarray(hidden_states, np.float32).reshape(T, E)
    hsT = np.ascontiguousarray(hs.T).astype(BF16)
    mask = np.asarray(attention_mask)
    padfull = np.where(mask != 0, 0.0, NEG).astype(np.float32)      # [B,S]
    pad = np.ascontiguousarray(
        padfull.reshape(B * 16, 128).T                               # [128, 32]
    )
    W_attn = np.asarray(W_attn, np.float32)
    W_proj = np.asarray(W_proj, np.float32)
    b_attn = np.asarray(b_attn, np.float32)

    in_maps = []
    for c in range(NCORE):
        rows = np.concatenate(
            [np.arange(sec * E + c * 128, sec * E + (c + 1) * 128)
             for sec in range(3)]
        )
        wq = np.ascontiguousarray(W_attn[rows].T).astype(BF16)       # [1024,384]
        bq = np.ascontiguousarray(b_attn[rows][None, :]).astype(BF16)
        wp = np.ascontiguousarray(W_proj[:, c * 128:(c + 1) * 128].T).astype(BF16)
        in_maps.append(
            {"hsT": hsT, "wqkv": wq, "bqkv": bq, "wpT": wp, "padneg": pad}
        )

    nc = _build()
    res = run_bass_kernel_spmd(nc, in_maps, list(range(NCORE)), trace=_trace)
    parts = np.stack([np.asarray(r["out"], np.float32) for r in res.results])
    outv = parts.sum(axis=0) + np.asarray(b_proj, np.float32)[None, :]
    out = outv.reshape(B, S, E).astype(np.float32)
    if _trace:
        return out, res
    return out



# revision 6
# speedup vs baseline: 1.1199x; 1.1199x over previous
"""GPT2 attention (B=2, S=2048, E=1024, H=16) on 8 NeuronCores.

Sharding: tensor-parallel over heads - 2 heads per core. Each core computes
qkv^T for its heads, causal attention in transposed-score layout (k on
partitions, q on free dim), then a partial output projection over its 128
ctx dims. Host sums the 8 partials and adds b_proj.

v2 design notes (vs the earlier 280us baseline):
- PV matmuls reoriented: lhsT=V (65/128 cols), rhs=exp(scores) with 512-wide
  free dim -> ctx^T emerges directly in [dim, token] layout, no final
  transposes, and the PE streams stay long (HAM clock stays at 2.4 GHz).
- Softmax denominator rides inside the PV matmul: vnatA col 64 (h0) / vnatB
  col 0 (h1) hold the attention-mask column, so psum row 64 (h0) / row 0
  (h1) accumulate sum(exp * mask). Pad masking is folded multiplicatively
  into V (exp bias = 0), which also lets one ACT instruction exponentiate
  both heads' score blocks ([128,1024] across 2 psum banks).
- Normalization: DVE reciprocal of the denominator rows, rank-1 matmul
  broadcast across partitions, DVE multiply into ctxT.
- QKV bias folded into the DVE psum-evacuation (tensor_scalar add).
- QKV tiles / v-transposes / output projection are interleaved into the
  attention loop as PE filler so the PE never idles long enough for the
  HAM activity monitor to throttle the clock.
"""
import os
from collections import deque

import numpy as np
import ml_dtypes

import concourse.bass as bass
import concourse.bacc as bacc
import concourse.tile as tile
from concourse import mybir
from concourse import masks
from concourse.bass_utils import run_bass_kernel_spmd

BF16 = ml_dtypes.bfloat16
B, S, E, H, D = 2, 2048, 1024, 16, 64
T = B * S                 # 4096 tokens
NCORE = 8
HPC = H // NCORE          # 2 heads per core
SCALE = D ** -0.5
F32 = mybir.dt.float32
BF = mybir.dt.bfloat16
EXP = mybir.ActivationFunctionType.Exp
CPY = mybir.ActivationFunctionType.Copy

_built = {}


def _build():
    if "nc" in _built:
        return _built["nc"]
    nc = bacc.Bacc()
    hsT = nc.declare_dram_parameter("hsT", [E, T], BF, isOutput=False)
    wqkv = nc.declare_dram_parameter("wqkv", [E, 3 * HPC * D], BF, isOutput=False)
    bqkv = nc.declare_dram_parameter("bqkv", [128, 3], F32, isOutput=False)
    wpT = nc.declare_dram_parameter("wpT", [HPC * D, E], BF, isOutput=False)
    maskv = nc.declare_dram_parameter("maskv", [128, 32, 1], BF, isOutput=False)
    masks_ = nc.declare_dram_parameter("masks", [128, 32], F32, isOutput=False)
    out = nc.declare_dram_parameter("out", [T, E], BF, isOutput=True)

    hsT_r = hsT.rearrange("(kc p) t -> kc p t", p=128)  # [8,128,4096]

    with tile.TileContext(nc) as tc:
        with (
            tc.tile_pool(name="const", bufs=1) as constp,
            tc.tile_pool(name="big", bufs=1) as bigp,
            tc.tile_pool(name="hst", bufs=16) as hstp,
            tc.tile_pool(name="vtmp", bufs=2) as vtmpp,
            tc.tile_pool(name="expt", bufs=4) as exptp,
            tc.tile_pool(name="outp", bufs=4) as outp,
            tc.tile_pool(name="bcs", bufs=2) as bcsp,
            tc.tile_pool(name="ps_fill", bufs=2, space="PSUM") as fillp,
            tc.tile_pool(name="ps_sc", bufs=2, space="PSUM") as scp_pool,
            tc.tile_pool(name="ps_ctx", bufs=2, space="PSUM") as ctxp,
        ):
            # ---- constants ----
            wqkv_sb = constp.tile([128, 8, 384], BF)
            nc.sync.dma_start(
                out=wqkv_sb, in_=wqkv.rearrange("(kc p) m -> p kc m", p=128)
            )
            bq_sb = constp.tile([128, 3], F32)
            nc.sync.dma_start(out=bq_sb, in_=bqkv[:])
            wpT_sb = constp.tile([128, E], BF)
            nc.sync.dma_start(out=wpT_sb, in_=wpT[:])
            msk_sb = constp.tile([128, 32], F32)
            nc.sync.dma_start(out=msk_sb, in_=masks_[:])
            ident = constp.tile([128, 128], BF)
            masks.make_identity(nc, ident[:])
            ones_sb = constp.tile([128, 64], BF)
            nc.vector.memset(ones_sb, 1.0)

            qT = bigp.tile([128, T], BF)       # rows: h0 dims 0-63, h1 dims 64-127
            kT = bigp.tile([128, T], BF)
            ctxT = bigp.tile([128, T], BF)
            # vnatA: per 128-token chunk: [h0 dims 0:64, mask col 64]
            # vnatB: per 128-token chunk: [mask col 0, zeros 1:64, h1 dims 64:128]
            vnatA = bigp.tile([128, 32, 65], BF)
            vnatB = bigp.tile([128, 32, 128], BF)
            rec = bigp.tile([128, 512], BF)
            nc.gpsimd.memset(vnatB[:, :, 1:64], 0.0)
            nc.sync.dma_start(out=vnatA[:, :, 64:65], in_=maskv[:])
            nc.sync.dma_start(out=vnatB[:, :, 0:1], in_=maskv[:])

            # ---- hsT tile prefetch ----
            hst_tiles = {}

            def prefetch(n):
                ts = []
                for k in range(8):
                    ht = hstp.tile([128, 512], BF, tag="ht", name=f"ht{n}_{k}")
                    nc.sync.dma_start(out=ht, in_=hsT_r[k, :, n * 512:(n + 1) * 512])
                    ts.append(ht)
                hst_tiles[n] = ts

            # ---- qkv tile for 512 tokens: filler units ----
            vtmp_of = {}

            def qkv_units(n):
                units = []
                pm = {}

                def mk_mm(m, klo, khi):
                    def u():
                        if klo == 0:
                            pm[m] = fillp.tile([128, 512], F32, tag="f",
                                               name=f"qkv{n}_{m}")
                        for k in range(klo, khi):
                            nc.tensor.matmul(
                                pm[m], lhsT=wqkv_sb[:, k, m * 128:(m + 1) * 128],
                                rhs=hst_tiles[n][k],
                                start=(k == 0), stop=(k == 7),
                            )
                    return u

                def mk_ev(m):
                    def u():
                        if m == 0:
                            nc.vector.tensor_scalar_add(
                                qT[:, n * 512:(n + 1) * 512], pm[0], bq_sb[:, 0:1])
                        elif m == 1:
                            nc.vector.tensor_scalar_add(
                                kT[:, n * 512:(n + 1) * 512], pm[1], bq_sb[:, 1:2])
                        else:
                            vt = vtmpp.tile([128, 512], BF, tag="vt",
                                            name=f"vt{n}")
                            nc.vector.tensor_scalar_add(vt, pm[2], bq_sb[:, 2:3])
                            vtmp_of[n] = vt
                    return u

                def mk_tr(t):
                    def u():
                        pst = fillp.tile([128, 128], BF, tag="f",
                                         name=f"tr{n}_{t}")
                        nc.tensor.transpose(
                            pst[:], vtmp_of[n][:, t * 128:(t + 1) * 128], ident[:])
                        tt4 = n * 4 + t
                        nc.vector.tensor_scalar_mul(
                            vnatA[:, tt4, 0:64], pst[:, 0:64],
                            msk_sb[:, tt4:tt4 + 1])
                        nc.vector.tensor_scalar_mul(
                            vnatB[:, tt4, 64:128], pst[:, 64:128],
                            msk_sb[:, tt4:tt4 + 1])
                    return u

                for m in range(3):
                    units.append(mk_mm(m, 0, 4))
                    units.append(mk_mm(m, 4, 8))
                    units.append(mk_ev(m))
                for t in range(4):
                    units.append(mk_tr(t))
                return units

            # ---- output projection for one qj block: filler units ----
            def proj_units(b, qj):
                units = []

                def mk_pj(t, n2):
                    tc_ = (4 * b + qj) * 4 + t

                    def u():
                        pp = fillp.tile([128, 512], F32, tag="f",
                                        name=f"pj{tc_}_{n2}")
                        nc.tensor.matmul(
                            pp, lhsT=ctxT[:, tc_ * 128:(tc_ + 1) * 128],
                            rhs=wpT_sb[:, n2 * 512:(n2 + 1) * 512],
                            start=True, stop=True,
                        )
                        ot = outp.tile([128, 512], BF, tag="ot")
                        if n2 == 0:
                            nc.vector.tensor_copy(ot, pp)
                        else:
                            nc.scalar.activation(out=ot, in_=pp, func=CPY)
                        nc.sync.dma_start(
                            out=out[tc_ * 128:(tc_ + 1) * 128,
                                    n2 * 512:(n2 + 1) * 512],
                            in_=ot,
                        )
                    return u

                for t in range(4):
                    for n2 in range(2):
                        units.append(mk_pj(t, n2))
                return units

            # ---- causal attention for one (b, qj) 512-query block ----
            def attention(b, qj, fq):
                nk = 4 * qj + 4
                ctxA = ctxp.tile([128, 512], F32, tag="ctx", name=f"cA{b}{qj}")
                ctxB = ctxp.tile([128, 512], F32, tag="ctx", name=f"cB{b}{qj}")
                qsl = slice(b * S + qj * 512, b * S + (qj + 1) * 512)
                exps = {}

                def pop_filler(k):
                    for _ in range(k):
                        if fq:
                            fq.popleft()()

                def emit_pv(ki):
                    e = exps.pop(ki)
                    kc = b * 16 + ki
                    nc.tensor.matmul(
                        ctxA[0:65, :], lhsT=vnatA[:, kc, :], rhs=e[:, 0:512],
                        start=(ki == 0), stop=(ki == nk - 1),
                    )
                    nc.tensor.matmul(
                        ctxB[:, :], lhsT=vnatB[:, kc, :], rhs=e[:, 512:1024],
                        start=(ki == 0), stop=(ki == nk - 1),
                    )

                for ki in range(nk):
                    scp = scp_pool.tile([128, 1024], F32, tag="sc",
                                        name=f"sc{b}{qj}_{ki}")
                    ksl = slice(b * S + ki * 128, b * S + (ki + 1) * 128)
                    nc.tensor.matmul(scp[:, 0:512], lhsT=kT[0:64, ksl],
                                     rhs=qT[0:64, qsl], start=True, stop=True)
                    nc.tensor.matmul(scp[:, 512:1024], lhsT=kT[64:128, ksl],
                                     rhs=qT[64:128, qsl], start=True, stop=True)
                    e = exptp.tile([128, 1024], BF, tag="e",
                                   name=f"ex{b}{qj}_{ki}")
                    nc.scalar.activation(out=e, in_=scp, func=EXP, scale=SCALE)
                    d = ki - 4 * qj
                    if d >= 0:   # diagonal: zero where k > q
                        for hh in range(2):
                            sl = e[:, hh * 512:(hh + 1) * 512]
                            nc.gpsimd.affine_select(
                                out=sl, in_=sl,
                                compare_op=mybir.AluOpType.is_ge, fill=0.0,
                                base=-(128 * d), channel_multiplier=-1,
                                pattern=[[1, 512]],
                            )
                    exps[ki] = e
                    slots_left = nk - ki + 1
                    pop_filler(-(-len(fq) // slots_left))
                    if ki >= 1:
                        emit_pv(ki - 1)
                pop_filler(len(fq))
                emit_pv(nk - 1)

                # normalize: h0 denom = ctxA row 64, h1 denom = ctxB row 0
                with nc.allow_low_precision(reason="bf16 recip feeds bf16 ctx"):
                    nc.vector.reciprocal(rec[64:65, :], ctxA[64:65, :])
                    nc.vector.reciprocal(rec[0:1, :], ctxB[0:1, :])
                bps = scp_pool.tile([128, 512], F32, tag="sc",
                                    name=f"bp{b}{qj}")
                nc.tensor.matmul(bps[0:64, :], lhsT=ones_sb[64:65, :],
                                 rhs=rec[64:65, :], start=True, stop=True)
                nc.tensor.matmul(bps[64:128, :], lhsT=ones_sb[0:1, :],
                                 rhs=rec[0:1, :], start=True, stop=True)
                bcs = bcsp.tile([128, 512], BF, tag="bc", name=f"bc{b}{qj}")
                nc.vector.tensor_copy(bcs, bps)
                nc.vector.tensor_mul(ctxT[0:64, qsl], ctxA[0:64, :],
                                     bcs[0:64, :])
                nc.vector.tensor_mul(ctxT[64:128, qsl], ctxB[64:128, :],
                                     bcs[64:128, :])

            # ---- main schedule ----
            prefetch(0)
            prefetch(1)
            for u in qkv_units(0):
                u()
            prev = None
            for b in range(B):
                for qj in range(4):
                    tt = 4 * b + qj
                    if tt + 2 <= 7:
                        prefetch(tt + 2)
                    fq = deque()
                    if tt + 1 <= 7:
                        fq.extend(qkv_units(tt + 1))
                    if prev is not None:
                        fq.extend(proj_units(*prev))
                    attention(b, qj, fq)
                    prev = (b, qj)
            for u in proj_units(1, 3):
                u()
    nc.finalize()
    _built["nc"] = nc
    return nc


def kernel(hidden_states, attention_mask, W_attn, b_attn, W_proj, b_proj,
           _trace=False):
    hs = np.asarray(hidden_states, np.float32).reshape(T, E)
    hsT = np.ascontiguousarray(hs.T).astype(BF16)
    mask = np.asarray(attention_mask)
    mcol = (mask.reshape(B * S) != 0).astype(np.float32)        # [4096]
    mchunk = np.ascontiguousarray(mcol.reshape(32, 128).T)       # [128, 32]
    maskv = mchunk[:, :, None].astype(BF16)
    masks_ = mchunk.astype(np.float32)
    W_attn = np.asarray(W_attn, np.float32)
    W_proj = np.asarray(W_proj, np.float32)
    b_attn = np.asarray(b_attn, np.float32)

    in_maps = []
    for c in range(NCORE):
        rows = np.concatenate(
            [np.arange(sec * E + c * 128, sec * E + (c + 1) * 128)
             for sec in range(3)]
        )
        wq = np.ascontiguousarray(W_attn[rows].T).astype(BF16)       # [1024,384]
        bq = np.ascontiguousarray(
            b_attn[rows].reshape(3, 128).T).astype(np.float32)             # [128,3] f32
        wp = np.ascontiguousarray(W_proj[:, c * 128:(c + 1) * 128].T).astype(BF16)
        in_maps.append(
            {"hsT": hsT, "wqkv": wq, "bqkv": bq, "wpT": wp,
             "maskv": maskv, "masks": masks_}
        )

    nc = _build()
    res = run_bass_kernel_spmd(nc, in_maps, list(range(NCORE)), trace=_trace)
    parts = np.stack([np.asarray(r["out"], np.float32) for r in res.results])
    outv = parts.sum(axis=0) + np.asarray(b_proj, np.float32)[None, :]
    out = outv.reshape(B, S, E).astype(np.float32)
    if _trace:
        return out, res
    return out
